# revision 1
# baseline (speedup 1.0000x reference)
"""Trainium2 Bass kernel for nn_Attention_78786880078481.

Full (unsharded) inputs in, full output out. Sharding: data-parallel over the
batch dim (B=8) across the 8 NeuronCores — one batch element per core, no
collectives needed.

Per-core computation (S=1024, NX=1024, H=16, HD=64), all matmuls in fp32r
(TF32) on the tensor engine:
  Window 1: PE-transpose x -> xT; kT = Wk.T @ xT (transposed layout);
     v = x @ Wv (natural layout, with a ones column per head for the softmax
     denominator).
  Window 1 also PE-transposes query -> qTin (kept resident).
  Window 2 (per 512-wide sq half, q-projection interleaved with attention so
  the exp() work on the scalar engine hides under tensor-engine matmuls):
     per 128-chunk m: qTmp = Wq[:,m].T @ qTin (a transient tile),
     and for the chunk's two heads: scoresT[sk, sq] = kT_h.T @ qTmp_h over
     causal block rows only, P = exp(scores/8) (no max-subtraction needed:
     |scores/8| < ~2; the reference's -1e4 mask bias underflows to exact 0
     after its max-subtracted softmax, so masked entries are exactly 0 there
     too), triangular-mask the diagonal blocks (gpsimd affine_select),
     attnT_aug[65, sq] = [v_h | 1].T @ P accumulated over sk chunks (row 64 =
     softmax denominator l), normalize with 1/l via DVE reciprocal + gpsimd
     partition broadcast (l hops to partition 0 by SBUF-to-SBUF DMA; odd heads
     reach their home partition offset 64 the same way).
  Phase D: stacked attnT = aT [NX, S] feeds c_proj directly: y = aT.T @ Wp.
Biases are zeros in setup_inputs(); bias matmuls are emitted only if nonzero.
"""

import sys

for p in ("/opt/trn_rl_repo",):
    if p not in sys.path:
        sys.path.insert(0, p)

import numpy as np

import concourse.bass as bass
import concourse.tile as tile
from concourse import bacc, mybir
from concourse.bass_utils import run_bass_kernel_spmd

F32 = mybir.dt.float32
F32R = mybir.dt.float32r
EXPF = mybir.ActivationFunctionType.Exp

_CACHE = {}
BUILD_MARKS = []  # (label, n_instructions) snapshots for profiling tools


def ceil_div(a, b):
    return (a + b - 1) // b


def build_module(S, NX, H, with_attn_bias, with_proj_bias, n_cores=8):
    """Build the per-core Bass module."""
    from contextlib import ExitStack

    HD = NX // H
    assert HD == 64, "kernel specialized for head_dim 64 (2 heads per 128-row chunk)"
    P = 128
    SB = S // P        # number of 128-row blocks of S
    KB = NX // P       # number of 128-deep contraction chunks over NX
    CW = min(512, S)   # column-tile width over S
    NQ = ceil_div(S, CW)
    NW = min(512, NX)  # column-tile width over NX
    NH = ceil_div(NX, NW)
    QW = min(256, NX)  # Wq streaming quarter width
    HC = HD + 1        # head stride in v_aug (v columns + ones column)
    scale = 1.0 / float(np.sqrt(HD))

    nc = bacc.Bacc("TRN2", target_bir_lowering=False, debug=False,
                   num_devices=n_cores)

    x_d = nc.dram_tensor("x", [S, NX], F32R, kind="ExternalInput")
    q_d = nc.dram_tensor("query", [S, NX], F32R, kind="ExternalInput")
    wa_d = nc.dram_tensor("c_attn_w", [NX, 3 * NX], F32R, kind="ExternalInput")
    wp_d = nc.dram_tensor("c_proj_w", [NX, NX], F32R, kind="ExternalInput")
    ident_d = nc.dram_tensor("ident", [P, P], F32R, kind="ExternalInput")
    mask_d = nc.dram_tensor("mask", [P, P], F32R, kind="ExternalInput")
    zeros_d = nc.dram_tensor("zeros", [P, P], F32R, kind="ExternalInput")
    ones_d = nc.dram_tensor("ones", [P, CW], F32R, kind="ExternalInput")
    if with_attn_bias:
        ba_d = nc.dram_tensor("c_attn_b", [1, 3 * NX], F32R, kind="ExternalInput")
    if with_proj_bias:
        bp_d = nc.dram_tensor("c_proj_b", [1, NX], F32R, kind="ExternalInput")
    out_d = nc.dram_tensor("out", [S, NX], F32, kind="ExternalOutput")

    BUILD_MARKS.clear()

    def mark(label):
        BUILD_MARKS.append((label, len(nc.inst_map)))

    with tile.TileContext(nc) as tc, ExitStack() as top:
        consts = top.enter_context(tc.tile_pool(name="consts", bufs=1))
        ident = consts.tile([P, P], F32R, tag="ident")
        mask = consts.tile([P, P], F32R, tag="mask")
        zeros = consts.tile([P, P], F32R, tag="zeros")
        ones = consts.tile([P, CW], F32R, tag="ones")
        nc.sync.dma_start(ident[:], ident_d[:])
        nc.sync.dma_start(mask[:], mask_d[:])
        nc.sync.dma_start(zeros[:], zeros_d[:])
        nc.sync.dma_start(ones[:], ones_d[:])
        if with_attn_bias:
            ba = consts.tile([1, 3 * NX], F32R, tag="ba")
            nc.sync.dma_start(ba[:], ba_d[:])
        if with_proj_bias:
            bp = consts.tile([1, NX], F32R, tag="bp")
            nc.sync.dma_start(bp[:], bp_d[:])

        # kT and v_aug live from window 1 through window 2.
        qkv = top.enter_context(tc.tile_pool(name="qkv", bufs=1))
        kT = qkv.tile([P, KB * S], F32R, tag="kT")      # [NX, S]
        v_aug = qkv.tile([P, SB * H * HC], F32R, tag="v")
        qTin_pool = top.enter_context(tc.tile_pool(name="qTin", bufs=1))
        qTin = qTin_pool.tile([P, KB * S], F32R, tag="qTin")

        def transpose_into(src_d, r0, r1, dstT, cstride, nat_pool, tp_pool):
            # dstT[:, cstride*k + (sb-r0)*P ...] = src[P*sb : .., P*k : ..].T
            for sb in range(r0, r1):
                nat = nat_pool.tile([P, NX], F32R, tag="nat")
                nc.sync.dma_start(nat[:], src_d[P * sb : P * (sb + 1), :])
                for k in range(KB):
                    ps = tp_pool.tile([P, P], F32R, tag="tp")
                    nc.tensor.transpose(
                        ps[:], nat[:, P * k : P * (k + 1)], ident[:]
                    )
                    dst = dstT[
                        :,
                        cstride * k + P * (sb - r0) : cstride * k + P * (sb - r0 + 1),
                    ]
                    if k % 2 == 0:
                        nc.vector.tensor_copy(dst, ps[:])
                    else:
                        nc.scalar.copy(dst, ps[:])

        # ---------------- Window 1: x side (kT, v) ----------------
        with ExitStack() as st_a:
            nat_pool = st_a.enter_context(tc.tile_pool(name="nat", bufs=3))
            tp_pool = st_a.enter_context(
                tc.tile_pool(name="tp", bufs=4, space="PSUM")
            )
            w_pool = st_a.enter_context(tc.tile_pool(name="w", bufs=2))
            pb_pool = st_a.enter_context(
                tc.tile_pool(name="pb", bufs=4, space="PSUM")
            )

            # ones columns of v_aug
            for sb in range(SB):
                va = v_aug[:, sb * H * HC : (sb + 1) * H * HC].rearrange(
                    "p (h c) -> p h c", c=HC
                )[:, :, HD : HD + 1]
                nc.vector.tensor_copy(
                    va, ones[:, :H].rearrange("p (h o) -> p h o", o=1)
                )
            mark("W1:setup")

            with tc.tile_pool(name="xT", bufs=1) as xT_pool:
                xT = xT_pool.tile([P, KB * S], F32R, tag="xT")
                transpose_into(x_d, 0, SB, xT, S, nat_pool, tp_pool)
                mark("W1:transposeX")

                # v in natural layout, scattered into v_aug
                for t in range(NH):
                    c0, c1 = NW * t, min(NW * (t + 1), NX)
                    w = w_pool.tile([P, KB * NW], F32R, tag="w")
                    for k in range(KB):
                        nc.sync.dma_start(
                            w[:, NW * k : NW * (k + 1)],
                            wa_d[P * k : P * (k + 1), 2 * NX + c0 : 2 * NX + c1],
                        )
                    for sb in range(SB):
                        ps = pb_pool.tile([P, CW], F32, tag="pb")
                        for k in range(KB):
                            nc.tensor.matmul(
                                ps[:, : c1 - c0],
                                xT[:, S * k + P * sb : S * k + P * (sb + 1)],
                                w[:, NW * k : NW * k + (c1 - c0)],
                                start=(k == 0),
                                stop=(k == KB - 1) and not with_attn_bias,
                            )
                        if with_attn_bias:
                            nc.tensor.matmul(
                                ps[:, : c1 - c0],
                                ones[0:1, :P],
                                ba[:, 2 * NX + c0 : 2 * NX + c1],
                                start=False,
                                stop=True,
                            )
                        h0 = c0 // HD
                        nh = (c1 - c0) // HD
                        dst = v_aug[
                            :,
                            sb * H * HC + h0 * HC : sb * H * HC + (h0 + nh) * HC,
                        ].rearrange("p (h c) -> p h c", c=HC)[:, :, :HD]
                        srcv = ps[:, : c1 - c0].rearrange("p (h c) -> p h c", c=HD)
                        if (sb + t) % 2 == 0:
                            nc.vector.tensor_copy(dst, srcv)
                        else:
                            nc.scalar.copy(dst, srcv)
                mark("W1:projV")
                transpose_into(q_d, 0, SB, qTin, S, nat_pool, tp_pool)
                mark("W1:transposeQ")
                # kT = Wk.T @ xT, W streamed in column halves
                for half in range(NX // NW):
                    w = w_pool.tile([P, KB * NW], F32R, tag="w")
                    for k in range(KB):
                        nc.sync.dma_start(
                            w[:, NW * k : NW * (k + 1)],
                            wa_d[
                                P * k : P * (k + 1),
                                NX + NW * half : NX + NW * (half + 1),
                            ],
                        )
                    for ml in range(NW // P):
                        m = half * (NW // P) + ml
                        for t in range(NQ):
                            c0, c1 = CW * t, min(CW * (t + 1), S)
                            ps = pb_pool.tile([P, CW], F32, tag="pb")
                            for k in range(KB):
                                nc.tensor.matmul(
                                    ps[:, : c1 - c0],
                                    w[:, NW * k + P * ml : NW * k + P * (ml + 1)],
                                    xT[:, S * k + c0 : S * k + c1],
                                    start=(k == 0),
                                    stop=(k == KB - 1) and not with_attn_bias,
                                )
                            if with_attn_bias:
                                nc.tensor.matmul(
                                    ps[:, : c1 - c0],
                                    ba[:, NX + P * m : NX + P * (m + 1)],
                                    ones[0:1, : c1 - c0],
                                    start=False,
                                    stop=True,
                                )
                            dst = kT[:, S * m + c0 : S * m + c1]
                            if (m + t) % 2 == 0:
                                nc.vector.tensor_copy(dst, ps[:, : c1 - c0])
                            else:
                                nc.scalar.copy(dst, ps[:, : c1 - c0])
                mark("W1:projK")

        # ------- Window 2: per sq half: transpose q, projQ + attention -------
        aT_pool = top.enter_context(tc.tile_pool(name="aTp", bufs=1))
        aT = aT_pool.tile([P, KB * S], F32R, tag="aT")

        def wlo_f(j, c0, c1):
            # widened column start: keep moving dim >= 256 (fp32r is 4x
            # slower below that); extra below-diagonal columns are zeroed.
            lo = max(c0, P * j)
            return max(c0, min(lo, c1 - 256)) if c1 - c0 >= 256 else lo

        def attn_scores(h, qTmp, c0, c1, pt_pool, sc_pool, j0, npair, PT):
            ch = h // 2
            po = HD * (h % 2)
            ps = sc_pool.tile([P, 2 * CW], F32, tag="sc")
            for j in range(j0, j0 + npair):
                lo2 = wlo_f(j, c0, c1)
                nc.tensor.matmul(
                    ps[:, CW * (j - j0) + (lo2 - c0) : CW * (j - j0) + (c1 - c0)],
                    kT[po : po + HD, S * ch + P * j : S * ch + P * (j + 1)],
                    qTmp[po : po + HD, lo2 - c0 : c1 - c0],
                    start=True,
                    stop=True,
                )
            # one exp over the chunk pair (the gap region holds garbage that
            # is never read back)
            lo0 = wlo_f(j0, c0, c1)
            hi = CW * (npair - 1) + (c1 - c0)
            nc.scalar.activation(
                PT[:, CW * j0 + (lo0 - c0) : CW * j0 + hi],
                ps[:, (lo0 - c0) : hi],
                EXPF,
                scale=scale,
            )
            for j in range(j0, j0 + npair):
                lo = max(c0, P * j)
                lo2 = wlo_f(j, c0, c1)
                if lo2 < lo:
                    # zero the widened below-diagonal columns (after exp)
                    nc.vector.tensor_copy(
                        PT[:, CW * j + (lo2 - c0) : CW * j + (lo - c0)],
                        zeros[:, : lo - lo2],
                    )
                if lo == P * j:
                    # diagonal block: zero strictly-lower (sk>sq)
                    d0 = CW * j + (lo - c0)
                    nc.vector.tensor_mul(
                        PT[:, d0 : d0 + P], PT[:, d0 : d0 + P], mask[:]
                    )

        def attn_tail(h, c0, c1, PT, at_pool, nrm_pool):
            ch = h // 2
            po = HD * (h % 2)
            W = c1 - c0
            jmax = min(SB, ceil_div(c1, P))
            # attnT_aug = [v_h | 1].T @ P : rows 0..HD attn, row HD = l
            psA = at_pool.tile([P, CW], F32, tag="at")
            for j in range(jmax):
                lo2 = wlo_f(j, c0, c1)
                nc.tensor.matmul(
                    psA[:HC, lo2 - c0 : W],
                    v_aug[:, (j * H + h) * HC : (j * H + h + 1) * HC],
                    PT[:, CW * j + (lo2 - c0) : CW * j + (c1 - c0)],
                    start=(j == 0),
                    stop=(j == jmax - 1),
                )
            rec = nrm_pool.tile([P, CW], F32, tag="rec")
            nc.vector.reciprocal(rec[HD : HD + 1, :W], psA[HD : HD + 1, :W])
            # hop 1/l to partition 0 (SBUF-to-SBUF DMA crosses partitions;
            # gpsimd broadcast only reads partition 0)
            nc.sync.dma_start(rec[0:1, :W], rec[HD : HD + 1, :W])
            bcs = nrm_pool.tile([P, CW], F32, tag="bcs")
            nc.gpsimd.partition_broadcast(bcs[:HD, :W], rec[0:1, :W])
            if po == 0:
                nc.vector.tensor_mul(
                    aT[:HD, S * ch + c0 : S * ch + c1],
                    psA[:HD, :W],
                    bcs[:HD, :W],
                )
            else:
                ash = nrm_pool.tile([P, CW], F32R, tag="ash")
                nc.vector.tensor_mul(ash[:HD, :W], psA[:HD, :W], bcs[:HD, :W])
                # partition shift 0->64 via SBUF-to-SBUF DMA
                nc.sync.dma_start(
                    aT[po : po + HD, S * ch + c0 : S * ch + c1], ash[:HD, :W]
                )

        def attn_pair(m, qTmp, c0, c1, pt_pool, sc_pool, at_pool, nrm_pool):
            # Interleave the two heads' K=64 score matmuls so they sit in
            # adjacent PE queue slots with disjoint row groups (partitions
            # 0-63 vs 64-127) -> the systolic array runs them concurrently.
            jmax = min(SB, ceil_div(c1, P))
            PTa = pt_pool.tile([P, SB * CW], F32R, tag="pt")
            PTb = pt_pool.tile([P, SB * CW], F32R, tag="pt")
            for j0 in range(0, jmax, 2):
                npair = min(2, jmax - j0)
                attn_scores(2 * m, qTmp, c0, c1, pt_pool, sc_pool, j0, npair, PTa)
                attn_scores(
                    2 * m + 1, qTmp, c0, c1, pt_pool, sc_pool, j0, npair, PTb
                )
            attn_tail(2 * m, c0, c1, PTa, at_pool, nrm_pool)
            attn_tail(2 * m + 1, c0, c1, PTb, at_pool, nrm_pool)

        with tc.tile_pool(name="wq", bufs=1) as wq_pool, tc.tile_pool(
            name="qTmp", bufs=3
        ) as qTmp_pool, tc.tile_pool(
            name="pbq", bufs=2, space="PSUM"
        ) as pbq_pool, tc.tile_pool(name="pt", bufs=3) as pt_pool, tc.tile_pool(
            name="sc", bufs=2, space="PSUM"
        ) as sc_pool, tc.tile_pool(
            name="at", bufs=2, space="PSUM"
        ) as at_pool, tc.tile_pool(name="nrm", bufs=2) as nrm_pool:
            for qtr in range(NX // QW):
                wq = wq_pool.tile([P, KB * QW], F32R, tag="wq")
                for k in range(KB):
                    nc.sync.dma_start(
                        wq[:, QW * k : QW * (k + 1)],
                        wa_d[P * k : P * (k + 1), QW * qtr : QW * (qtr + 1)],
                    )
                for ml in range(QW // P):
                    m = qtr * (QW // P) + ml
                    # t-inner: interleave the light (chain-bound) low-sq
                    # units with the heavy high-sq units for better packing,
                    # and load each Wq quarter once instead of once per half.
                    for t in range(NQ):
                        c0, c1 = CW * t, min(CW * (t + 1), S)
                        Wt = c1 - c0
                        ps = pbq_pool.tile([P, CW], F32, tag="pbq")
                        for k in range(KB):
                            nc.tensor.matmul(
                                ps[:, :Wt],
                                wq[:, QW * k + P * ml : QW * k + P * (ml + 1)],
                                qTin[:, S * k + c0 : S * k + c1],
                                start=(k == 0),
                                stop=(k == KB - 1) and not with_attn_bias,
                            )
                        if with_attn_bias:
                            nc.tensor.matmul(
                                ps[:, :Wt],
                                ba[:, P * m : P * (m + 1)],
                                ones[0:1, :Wt],
                                start=False,
                                stop=True,
                            )
                        qTmp = qTmp_pool.tile([P, CW], F32R, tag="qTmp")
                        if (m + t) % 2 == 0:
                            nc.vector.tensor_copy(qTmp[:, :Wt], ps[:, :Wt])
                        else:
                            nc.scalar.copy(qTmp[:, :Wt], ps[:, :Wt])
                        attn_pair(
                            m, qTmp, c0, c1,
                            pt_pool, sc_pool, at_pool, nrm_pool,
                        )
                mark(f"W2:qtr{qtr}")

        # ---------------- Phase D: output projection ----------------
        with tc.tile_pool(name="wp", bufs=1) as wp_pool, tc.tile_pool(
            name="pd", bufs=4, space="PSUM"
        ) as pd_pool, tc.tile_pool(name="yo", bufs=3) as yo_pool:
            wp = wp_pool.tile([P, KB * NX], F32R, tag="wp")
            for k in range(KB):
                nc.sync.dma_start(
                    wp[:, NX * k : NX * (k + 1)], wp_d[P * k : P * (k + 1), :]
                )
            for sb in range(SB):
                for t in range(NH):
                    c0, c1 = NW * t, min(NW * (t + 1), NX)
                    ps = pd_pool.tile([P, NW], F32, tag="pd")
                    for k in range(KB):
                        nc.tensor.matmul(
                            ps[:, : c1 - c0],
                            aT[:, S * k + P * sb : S * k + P * (sb + 1)],
                            wp[:, NX * k + c0 : NX * k + c1],
                            start=(k == 0),
                            stop=(k == KB - 1) and not with_proj_bias,
                        )
                    if with_proj_bias:
                        nc.tensor.matmul(
                            ps[:, : c1 - c0],
                            ones[0:1, :P],
                            bp[:, c0:c1],
                            start=False,
                            stop=True,
                        )
                    yo = yo_pool.tile([P, NW], F32, tag="yo")
                    if (sb + t) % 2 == 0:
                        nc.vector.tensor_copy(yo[:, : c1 - c0], ps[:, : c1 - c0])
                    else:
                        nc.scalar.copy(yo[:, : c1 - c0], ps[:, : c1 - c0])
                    nc.sync.dma_start(
                        out_d[P * sb : P * (sb + 1), c0:c1], yo[:, : c1 - c0]
                    )
        mark("D:proj")

    nc.compile()
    return nc


def get_module(S, NX, H, with_attn_bias, with_proj_bias, n_cores=8):
    key = (S, NX, H, with_attn_bias, with_proj_bias, n_cores)
    if key not in _CACHE:
        _CACHE[key] = build_module(
            S, NX, H, with_attn_bias, with_proj_bias, n_cores
        )
    return _CACHE[key]


def make_const_inputs(S, NX):
    P = 128
    CW = min(512, S)
    return {
        "ident": np.eye(P, dtype=np.float32),
        # mask[sk, sq] = 1 where sk <= sq (upper triangular incl diagonal)
        "mask": np.triu(np.ones((P, P), dtype=np.float32)),
        "zeros": np.zeros((P, P), dtype=np.float32),
        "ones": np.ones((P, CW), dtype=np.float32),
    }


def kernel(x, query, c_attn_w, c_attn_b, c_proj_w, c_proj_b, _trace=False):
    x = np.ascontiguousarray(np.asarray(x, dtype=np.float32))
    query = np.ascontiguousarray(np.asarray(query, dtype=np.float32))
    c_attn_w = np.ascontiguousarray(np.asarray(c_attn_w, dtype=np.float32))
    c_attn_b = np.asarray(c_attn_b, dtype=np.float32)
    c_proj_w = np.ascontiguousarray(np.asarray(c_proj_w, dtype=np.float32))
    c_proj_b = np.asarray(c_proj_b, dtype=np.float32)

    B, S, NX = x.shape
    H = 16
    wab = bool(np.any(c_attn_b != 0))
    wpb = bool(np.any(c_proj_b != 0))
    n_cores = 8
    nc = get_module(S, NX, H, wab, wpb, n_cores)

    base = make_const_inputs(S, NX)
    base["c_attn_w"] = c_attn_w
    base["c_proj_w"] = c_proj_w
    if wab:
        base["c_attn_b"] = np.ascontiguousarray(c_attn_b.reshape(1, -1))
    if wpb:
        base["c_proj_b"] = np.ascontiguousarray(c_proj_b.reshape(1, -1))

    in_maps = []
    for c in range(n_cores):
        m = dict(base)
        m["x"] = x[c % B]
        m["query"] = query[c % B]
        in_maps.append(m)

    res = run_bass_kernel_spmd(
        nc, in_maps, core_ids=list(range(n_cores)), trace=_trace
    )
    out = np.stack([res.results[c]["out"] for c in range(B)], axis=0)
    if _trace:
        kernel._last_results = res
    return out



# revision 7
# speedup vs baseline: 79.1771x; 79.1771x over previous
"""Trainium2 Bass kernel for nn_Attention_78786880078481.

Full (unsharded) inputs in, full output out. Sharding: data-parallel over the
batch dim (B=8) across the 8 NeuronCores — one batch element per core, no
collectives needed.

v2: bf16 end-to-end on the matmul datapath (inputs/weights converted to bf16
on the host, halving HBM traffic, SBUF footprint and PSUM->SBUF copy widths;
PSUM accumulation stays fp32), exact-causal score tiles (the fp32r
"keep moving dim >= 256" widening is unnecessary for bf16, which runs at
1 cycle/row at any width), per-block exp (no garbage-gap exp), and a
softmax tail unchanged from v1 (DVE reciprocal, SBUF-to-SBUF DMA hop to
partition 0, gpsimd partition_broadcast, DVE multiply): tensor_tensor cannot
read two PSUM operands, which rules out a PE-matmul-based broadcast.

Per-core computation (S=1024, NX=1024, H=16, HD=64):
  Window 1: PE-transpose x -> xT (bf16, 4 transposes batched per PSUM bank so
     each PSUM->SBUF copy moves [128,512]); v = x @ Wv augmented with a ones
     column per head (softmax denominator); PE-transpose query -> qTin;
     kT = Wk.T @ xT.
  Window 2 (per 512-wide sq half, q-projection interleaved with attention):
     per 128-chunk m: qTmp = Wq[:,m].T @ qTin; for the chunk's two heads:
     scoresT[sk, sq] = kT_h.T @ qTmp_h over causal block rows only,
     P = exp(scores/8) per block (no max-subtraction needed: |scores/8| < ~2;
     the reference's -1e4 mask bias underflows to exact 0 after its
     max-subtracted softmax, so masked entries are exactly 0 there too),
     triangular-mask the diagonal blocks, attnT_aug[65, sq] = [v_h | 1].T @ P
     accumulated over sk chunks (row 64 = softmax denominator l), then
     1/l via DVE reciprocal + gpsimd partition broadcast, one DVE multiply
     into aT (odd heads route via an SBUF staging tile + partition-shift DMA
     to their home offset 64).
  Phase D: stacked attnT = aT [NX, S] feeds c_proj directly: y = aT.T @ Wp.
Biases are zeros in setup_inputs(); bias matmuls are emitted only if nonzero.
"""

import sys

for p in ("/opt/trn_rl_repo",):
    if p not in sys.path:
        sys.path.insert(0, p)

import numpy as np
import ml_dtypes

import concourse.bass as bass
import concourse.tile as tile
from concourse import bacc, mybir
from concourse.bass_utils import run_bass_kernel_spmd

F32 = mybir.dt.float32
F32R = mybir.dt.float32r
BF16 = mybir.dt.bfloat16
EXPF = mybir.ActivationFunctionType.Exp
NP_BF16 = ml_dtypes.bfloat16

_CACHE = {}
BUILD_MARKS = []  # (label, n_instructions) snapshots for profiling tools


def ceil_div(a, b):
    return (a + b - 1) // b


def build_module(S, NX, H, with_attn_bias, with_proj_bias, n_cores=8):
    """Build the per-core Bass module."""
    from contextlib import ExitStack

    HD = NX // H
    assert HD == 64, "kernel specialized for head_dim 64 (2 heads per 128-row chunk)"
    P = 128
    SB = S // P        # number of 128-row blocks of S
    KB = NX // P       # number of 128-deep contraction chunks over NX
    CW = min(512, S)   # column-tile width over S
    NQ = ceil_div(S, CW)
    NW = min(512, NX)  # column-tile width over NX
    NH = ceil_div(NX, NW)
    QW = min(256, NX)  # Wq streaming quarter width
    HC = HD + 1        # head stride in v_aug (v columns + ones column)
    scale = 1.0 / float(np.sqrt(HD))

    nc = bacc.Bacc("TRN2", target_bir_lowering=False, debug=False,
                   num_devices=n_cores)

    x_d = nc.dram_tensor("x", [S, NX], BF16, kind="ExternalInput")
    q_d = nc.dram_tensor("query", [S, NX], BF16, kind="ExternalInput")
    wa_d = nc.dram_tensor("c_attn_w", [NX, 3 * NX], BF16, kind="ExternalInput")
    wp_d = nc.dram_tensor("c_proj_w", [NX, NX], BF16, kind="ExternalInput")
    ident_d = nc.dram_tensor("ident", [P, P], BF16, kind="ExternalInput")
    mask_d = nc.dram_tensor("mask", [P, P], BF16, kind="ExternalInput")
    ones_d = nc.dram_tensor("ones", [P, CW], BF16, kind="ExternalInput")
    if with_attn_bias:
        ba_d = nc.dram_tensor("c_attn_b", [1, 3 * NX], BF16, kind="ExternalInput")
    if with_proj_bias:
        bp_d = nc.dram_tensor("c_proj_b", [1, NX], BF16, kind="ExternalInput")
    out_d = nc.dram_tensor("out", [S, NX], F32, kind="ExternalOutput")

    BUILD_MARKS.clear()

    def mark(label):
        BUILD_MARKS.append((label, len(nc.inst_map)))

    with tile.TileContext(nc) as tc, ExitStack() as top:
        consts = top.enter_context(tc.tile_pool(name="consts", bufs=1))
        ident = consts.tile([P, P], BF16, tag="ident")
        mask = consts.tile([P, P], BF16, tag="mask")
        ones = consts.tile([P, CW], BF16, tag="ones")
        nc.sync.dma_start(ident[:], ident_d[:])
        nc.sync.dma_start(mask[:], mask_d[:])
        nc.sync.dma_start(ones[:], ones_d[:])
        if with_attn_bias:
            ba = consts.tile([1, 3 * NX], BF16, tag="ba")
            nc.sync.dma_start(ba[:], ba_d[:])
        if with_proj_bias:
            bp = consts.tile([1, NX], BF16, tag="bp")
            nc.sync.dma_start(bp[:], bp_d[:])

        # kT and v_aug live from window 1 through window 2.
        qkv = top.enter_context(tc.tile_pool(name="qkv", bufs=1))
        kT = qkv.tile([P, KB * S], BF16, tag="kT")      # [NX, S]
        v_aug = qkv.tile([P, SB * H * HC], BF16, tag="v")
        qTin_pool = top.enter_context(tc.tile_pool(name="qTin", bufs=1))
        qTin = qTin_pool.tile([P, KB * S], BF16, tag="qTin")

        def transpose_into(src_d, r0, r1, dstT, nat_pool, tp_pool):
            # dstT[:, S*k + (sb-r0)*P ...] = src[P*sb : .., P*k : ..].T
            # 4 transposes share one PSUM tile -> one [128, 4*128] copy.
            dview = dstT.rearrange("p (k s) -> p k s", s=S)
            for sb in range(r0, r1):
                nat = nat_pool.tile([P, NX], BF16, tag="nat")
                nc.sync.dma_start(nat[:], src_d[P * sb : P * (sb + 1), :])
                for k0 in range(0, KB, 4):
                    ps = tp_pool.tile([P, 4 * P], BF16, tag="tp")
                    psv = ps.rearrange("p (k s) -> p k s", s=P)
                    for k in range(k0, k0 + 4):
                        nc.tensor.transpose(
                            psv[:, k - k0, :], nat[:, P * k : P * (k + 1)],
                            ident[:],
                        )
                    dst = dview[
                        :, k0 : k0 + 4, P * (sb - r0) : P * (sb - r0 + 1)
                    ]
                    if (sb + k0) % 8 < 4:
                        nc.vector.tensor_copy(dst, psv[:])
                    else:
                        nc.scalar.copy(dst, psv[:])

        # ---------------- Window 1: x side (kT, v) ----------------
        with ExitStack() as st_a:
            nat_pool = st_a.enter_context(tc.tile_pool(name="nat", bufs=3))
            tp_pool = st_a.enter_context(
                tc.tile_pool(name="tp", bufs=4, space="PSUM")
            )
            w_pool = st_a.enter_context(tc.tile_pool(name="w", bufs=2))
            pb_pool = st_a.enter_context(
                tc.tile_pool(name="pb", bufs=4, space="PSUM")
            )

            # ones columns of v_aug
            for sb in range(SB):
                va = v_aug[:, sb * H * HC : (sb + 1) * H * HC].rearrange(
                    "p (h c) -> p h c", c=HC
                )[:, :, HD : HD + 1]
                nc.vector.tensor_copy(
                    va, ones[:, :H].rearrange("p (h o) -> p h o", o=1)
                )
            mark("W1:setup")

            with tc.tile_pool(name="xT", bufs=1) as xT_pool:
                xT = xT_pool.tile([P, KB * S], BF16, tag="xT")
                transpose_into(x_d, 0, SB, xT, nat_pool, tp_pool)
                mark("W1:transposeX")

                # v in natural layout, scattered into v_aug
                for t in range(NH):
                    c0, c1 = NW * t, min(NW * (t + 1), NX)
                    w = w_pool.tile([P, KB * NW], BF16, tag="w")
                    for k in range(KB):
                        nc.sync.dma_start(
                            w[:, NW * k : NW * (k + 1)],
                            wa_d[P * k : P * (k + 1), 2 * NX + c0 : 2 * NX + c1],
                        )
                    for sb in range(SB):
                        ps = pb_pool.tile([P, CW], F32, tag="pb")
                        for k in range(KB):
                            nc.tensor.matmul(
                                ps[:, : c1 - c0],
                                xT[:, S * k + P * sb : S * k + P * (sb + 1)],
                                w[:, NW * k : NW * k + (c1 - c0)],
                                start=(k == 0),
                                stop=(k == KB - 1) and not with_attn_bias,
                            )
                        if with_attn_bias:
                            nc.tensor.matmul(
                                ps[:, : c1 - c0],
                                ones[0:1, :P],
                                ba[:, 2 * NX + c0 : 2 * NX + c1],
                                start=False,
                                stop=True,
                            )
                        h0 = c0 // HD
                        nh = (c1 - c0) // HD
                        dst = v_aug[
                            :,
                            sb * H * HC + h0 * HC : sb * H * HC + (h0 + nh) * HC,
                        ].rearrange("p (h c) -> p h c", c=HC)[:, :, :HD]
                        srcv = ps[:, : c1 - c0].rearrange("p (h c) -> p h c", c=HD)
                        if (sb + t) % 2 == 0:
                            nc.vector.tensor_copy(dst, srcv)
                        else:
                            nc.scalar.copy(dst, srcv)
                mark("W1:projV")
                transpose_into(q_d, 0, SB, qTin, nat_pool, tp_pool)
                mark("W1:transposeQ")
                # kT = Wk.T @ xT, W streamed in column halves
                for half in range(NX // NW):
                    w = w_pool.tile([P, KB * NW], BF16, tag="w")
                    for k in range(KB):
                        nc.sync.dma_start(
                            w[:, NW * k : NW * (k + 1)],
                            wa_d[
                                P * k : P * (k + 1),
                                NX + NW * half : NX + NW * (half + 1),
                            ],
                        )
                    for ml in range(NW // P):
                        m = half * (NW // P) + ml
                        for t in range(NQ):
                            c0, c1 = CW * t, min(CW * (t + 1), S)
                            ps = pb_pool.tile([P, CW], F32, tag="pb")
                            for k in range(KB):
                                nc.tensor.matmul(
                                    ps[:, : c1 - c0],
                                    w[:, NW * k + P * ml : NW * k + P * (ml + 1)],
                                    xT[:, S * k + c0 : S * k + c1],
                                    start=(k == 0),
                                    stop=(k == KB - 1) and not with_attn_bias,
                                )
                            if with_attn_bias:
                                nc.tensor.matmul(
                                    ps[:, : c1 - c0],
                                    ba[:, NX + P * m : NX + P * (m + 1)],
                                    ones[0:1, : c1 - c0],
                                    start=False,
                                    stop=True,
                                )
                            dst = kT[:, S * m + c0 : S * m + c1]
                            if (m + t) % 2 == 0:
                                nc.vector.tensor_copy(dst, ps[:, : c1 - c0])
                            else:
                                nc.scalar.copy(dst, ps[:, : c1 - c0])
                mark("W1:projK")

        # ------- Window 2: per sq half: projQ + attention -------
        aT_pool = top.enter_context(tc.tile_pool(name="aTp", bufs=1))
        aT = aT_pool.tile([P, KB * S], BF16, tag="aT")

        def attn_scores(h, qTmp, c0, c1, sc_pool, j0, npair, PT):
            ch = h // 2
            po = HD * (h % 2)
            ps = sc_pool.tile([P, 2 * CW], F32, tag="sc")
            for j in range(j0, j0 + npair):
                lo = max(c0, P * j)
                nc.tensor.matmul(
                    ps[:, CW * (j - j0) + (lo - c0) : CW * (j - j0) + (c1 - c0)],
                    kT[po : po + HD, S * ch + P * j : S * ch + P * (j + 1)],
                    qTmp[po : po + HD, lo - c0 : c1 - c0],
                    start=True,
                    stop=True,
                )
            # exp per block over the exact causal region
            for j in range(j0, j0 + npair):
                lo = max(c0, P * j)
                nc.scalar.activation(
                    PT[:, CW * j + (lo - c0) : CW * j + (c1 - c0)],
                    ps[:, CW * (j - j0) + (lo - c0) : CW * (j - j0) + (c1 - c0)],
                    EXPF,
                    scale=scale,
                )
                if lo == P * j:
                    # diagonal block: zero strictly-lower (sk>sq)
                    d0 = CW * j + (lo - c0)
                    nc.vector.tensor_mul(
                        PT[:, d0 : d0 + P], PT[:, d0 : d0 + P], mask[:]
                    )

        def attn_tail(h, c0, c1, PT, at_pool, nrm_pool):
            ch = h // 2
            po = HD * (h % 2)
            W = c1 - c0
            jmax = min(SB, ceil_div(c1, P))
            # attnT_aug = [v_h | 1].T @ P : rows 0..HD attn, row HD = l
            psA = at_pool.tile([P, CW], F32, tag="at")
            for j in range(jmax):
                lo = max(c0, P * j)
                nc.tensor.matmul(
                    psA[:HC, lo - c0 : W],
                    v_aug[:, (j * H + h) * HC : (j * H + h + 1) * HC],
                    PT[:, CW * j + (lo - c0) : CW * j + (c1 - c0)],
                    start=(j == 0),
                    stop=(j == jmax - 1),
                )
            rec = nrm_pool.tile([P, CW], F32, tag="rec")
            nc.vector.reciprocal(rec[HD : HD + 1, :W], psA[HD : HD + 1, :W])
            # hop 1/l to partition 0 (SBUF-to-SBUF DMA crosses partitions;
            # gpsimd broadcast only reads partition 0)
            nc.sync.dma_start(rec[0:1, :W], rec[HD : HD + 1, :W])
            bcs = nrm_pool.tile([P, CW], F32, tag="bcs")
            nc.gpsimd.partition_broadcast(bcs[:HD, :W], rec[0:1, :W])
            if po == 0:
                nc.vector.tensor_mul(
                    aT[:HD, S * ch + c0 : S * ch + c1],
                    psA[:HD, :W],
                    bcs[:HD, :W],
                )
            else:
                ash = nrm_pool.tile([P, CW], BF16, tag="ash")
                nc.vector.tensor_mul(ash[:HD, :W], psA[:HD, :W], bcs[:HD, :W])
                # partition shift 0->64 via SBUF-to-SBUF DMA
                nc.sync.dma_start(
                    aT[po : po + HD, S * ch + c0 : S * ch + c1], ash[:HD, :W]
                )

        def attn_pair(m, qTmp, c0, c1, pt_pool, sc_pool, at_pool, nrm_pool):
            # Interleave the two heads' K=64 score matmuls so they sit in
            # adjacent PE queue slots with disjoint row groups (partitions
            # 0-63 vs 64-127) -> the systolic array runs them concurrently.
            jmax = min(SB, ceil_div(c1, P))
            PTa = pt_pool.tile([P, SB * CW], BF16, tag="pt")
            PTb = pt_pool.tile([P, SB * CW], BF16, tag="pt")
            for j0 in range(0, jmax, 2):
                npair = min(2, jmax - j0)
                attn_scores(2 * m, qTmp, c0, c1, sc_pool, j0, npair, PTa)
                attn_scores(2 * m + 1, qTmp, c0, c1, sc_pool, j0, npair, PTb)
            attn_tail(2 * m, c0, c1, PTa, at_pool, nrm_pool)
            attn_tail(2 * m + 1, c0, c1, PTb, at_pool, nrm_pool)

        with tc.tile_pool(name="wq", bufs=1) as wq_pool, tc.tile_pool(
            name="qTmp", bufs=3
        ) as qTmp_pool, tc.tile_pool(
            name="pbq", bufs=2, space="PSUM"
        ) as pbq_pool, tc.tile_pool(name="pt", bufs=3) as pt_pool, tc.tile_pool(
            name="sc", bufs=2, space="PSUM"
        ) as sc_pool, tc.tile_pool(
            name="at", bufs=2, space="PSUM"
        ) as at_pool, tc.tile_pool(name="nrm", bufs=2) as nrm_pool:
            for qtr in range(NX // QW):
                wq = wq_pool.tile([P, KB * QW], BF16, tag="wq")
                for k in range(KB):
                    nc.sync.dma_start(
                        wq[:, QW * k : QW * (k + 1)],
                        wa_d[P * k : P * (k + 1), QW * qtr : QW * (qtr + 1)],
                    )
                for ml in range(QW // P):
                    m = qtr * (QW // P) + ml
                    # t-inner: interleave the light (chain-bound) low-sq
                    # units with the heavy high-sq units for better packing,
                    # and load each Wq quarter once instead of once per half.
                    for t in range(NQ):
                        c0, c1 = CW * t, min(CW * (t + 1), S)
                        Wt = c1 - c0
                        ps = pbq_pool.tile([P, CW], F32, tag="pbq")
                        for k in range(KB):
                            nc.tensor.matmul(
                                ps[:, :Wt],
                                wq[:, QW * k + P * ml : QW * k + P * (ml + 1)],
                                qTin[:, S * k + c0 : S * k + c1],
                                start=(k == 0),
                                stop=(k == KB - 1) and not with_attn_bias,
                            )
                        if with_attn_bias:
                            nc.tensor.matmul(
                                ps[:, :Wt],
                                ba[:, P * m : P * (m + 1)],
                                ones[0:1, :Wt],
                                start=False,
                                stop=True,
                            )
                        qTmp = qTmp_pool.tile([P, CW], BF16, tag="qTmp")
                        # DVE: the ACT engine is exp-saturated in window 2
                        nc.vector.tensor_copy(qTmp[:, :Wt], ps[:, :Wt])
                        attn_pair(
                            m, qTmp, c0, c1,
                            pt_pool, sc_pool, at_pool, nrm_pool,
                        )
                mark(f"W2:qtr{qtr}")

        # ---------------- Phase D: output projection ----------------
        with tc.tile_pool(name="wp", bufs=1) as wp_pool, tc.tile_pool(
            name="pd", bufs=4, space="PSUM"
        ) as pd_pool, tc.tile_pool(name="yo", bufs=3) as yo_pool:
            wp = wp_pool.tile([P, KB * NX], BF16, tag="wp")
            for k in range(KB):
                nc.sync.dma_start(
                    wp[:, NX * k : NX * (k + 1)], wp_d[P * k : P * (k + 1), :]
                )
            for sb in range(SB):
                for t in range(NH):
                    c0, c1 = NW * t, min(NW * (t + 1), NX)
                    ps = pd_pool.tile([P, NW], F32, tag="pd")
                    for k in range(KB):
                        nc.tensor.matmul(
                            ps[:, : c1 - c0],
                            aT[:, S * k + P * sb : S * k + P * (sb + 1)],
                            wp[:, NX * k + c0 : NX * k + c1],
                            start=(k == 0),
                            stop=(k == KB - 1) and not with_proj_bias,
                        )
                    if with_proj_bias:
                        nc.tensor.matmul(
                            ps[:, : c1 - c0],
                            ones[0:1, :P],
                            bp[:, c0:c1],
                            start=False,
                            stop=True,
                        )
                    yo = yo_pool.tile([P, NW], F32, tag="yo")
                    if (sb + t) % 2 == 0:
                        nc.vector.tensor_copy(yo[:, : c1 - c0], ps[:, : c1 - c0])
                    else:
                        nc.scalar.copy(yo[:, : c1 - c0], ps[:, : c1 - c0])
                    nc.sync.dma_start(
                        out_d[P * sb : P * (sb + 1), c0:c1], yo[:, : c1 - c0]
                    )
        mark("D:proj")

    nc.compile()
    return nc


def get_module(S, NX, H, with_attn_bias, with_proj_bias, n_cores=8):
    key = (S, NX, H, with_attn_bias, with_proj_bias, n_cores)
    if key not in _CACHE:
        _CACHE[key] = build_module(
            S, NX, H, with_attn_bias, with_proj_bias, n_cores
        )
    return _CACHE[key]


def make_const_inputs(S, NX):
    P = 128
    CW = min(512, S)
    HD = 64
    return {
        "ident": np.eye(P, dtype=NP_BF16),
        # mask[sk, sq] = 1 where sk <= sq (upper triangular incl diagonal)
        "mask": np.triu(np.ones((P, P), dtype=NP_BF16)),
        "ones": np.ones((P, CW), dtype=NP_BF16),
    }


def kernel(x, query, c_attn_w, c_attn_b, c_proj_w, c_proj_b, _trace=False):
    x = np.ascontiguousarray(np.asarray(x, dtype=np.float32)).astype(NP_BF16)
    query = np.ascontiguousarray(
        np.asarray(query, dtype=np.float32)
    ).astype(NP_BF16)
    c_attn_w = np.ascontiguousarray(
        np.asarray(c_attn_w, dtype=np.float32)
    ).astype(NP_BF16)
    c_attn_b = np.asarray(c_attn_b, dtype=np.float32)
    c_proj_w = np.ascontiguousarray(
        np.asarray(c_proj_w, dtype=np.float32)
    ).astype(NP_BF16)
    c_proj_b = np.asarray(c_proj_b, dtype=np.float32)

    B, S, NX = x.shape
    H = 16
    wab = bool(np.any(c_attn_b != 0))
    wpb = bool(np.any(c_proj_b != 0))
    n_cores = 8
    nc = get_module(S, NX, H, wab, wpb, n_cores)

    base = make_const_inputs(S, NX)
    base["c_attn_w"] = c_attn_w
    base["c_proj_w"] = c_proj_w
    if wab:
        base["c_attn_b"] = np.ascontiguousarray(
            c_attn_b.reshape(1, -1)
        ).astype(NP_BF16)
    if wpb:
        base["c_proj_b"] = np.ascontiguousarray(
            c_proj_b.reshape(1, -1)
        ).astype(NP_BF16)

    in_maps = []
    for c in range(n_cores):
        m = dict(base)
        m["x"] = x[c % B]
        m["query"] = query[c % B]
        in_maps.append(m)

    res = run_bass_kernel_spmd(
        nc, in_maps, core_ids=list(range(n_cores)), trace=_trace
    )
    out = np.stack([res.results[c]["out"] for c in range(B)], axis=0)
    if _trace:
        kernel._last_results = res
    return out


# revision 10
# speedup vs baseline: 109.4337x; 1.3821x over previous
"""Trainium2 Bass kernel for nn_Attention_78786880078481.

Full (unsharded) inputs in, full output out. Sharding: data-parallel over the
batch dim (B=8) across the 8 NeuronCores — one batch element per core, no
collectives needed.

v2: bf16 end-to-end on the matmul datapath (inputs/weights converted to bf16
on the host, halving HBM traffic, SBUF footprint and PSUM->SBUF copy widths;
PSUM accumulation stays fp32), exact-causal score tiles (the fp32r
"keep moving dim >= 256" widening is unnecessary for bf16, which runs at
1 cycle/row at any width), per-block exp (no garbage-gap exp), and a
softmax tail unchanged from v1 (DVE reciprocal, SBUF-to-SBUF DMA hop to
partition 0, gpsimd partition_broadcast, DVE multiply): tensor_tensor cannot
read two PSUM operands, which rules out a PE-matmul-based broadcast.

Per-core computation (S=1024, NX=1024, H=16, HD=64):
  Window 1: PE-transpose x -> xT (bf16, 4 transposes batched per PSUM bank so
     each PSUM->SBUF copy moves [128,512]); v = x @ Wv augmented with a ones
     column per head (softmax denominator); PE-transpose query -> qTin;
     kT = Wk.T @ xT.
  Window 2 (per 512-wide sq half, q-projection interleaved with attention):
     per 128-chunk m: qTmp = Wq[:,m].T @ qTin; for the chunk's two heads:
     scoresT[sk, sq] = kT_h.T @ qTmp_h over causal block rows only,
     P = exp(scores/8) per block (no max-subtraction needed: |scores/8| < ~2;
     the reference's -1e4 mask bias underflows to exact 0 after its
     max-subtracted softmax, so masked entries are exactly 0 there too),
     triangular-mask the diagonal blocks, attnT_aug[65, sq] = [v_h | 1].T @ P
     accumulated over sk chunks (row 64 = softmax denominator l), then
     1/l via DVE reciprocal + gpsimd partition broadcast, one DVE multiply
     into aT (odd heads route via an SBUF staging tile + partition-shift DMA
     to their home offset 64).
  Phase D: stacked attnT = aT [NX, S] feeds c_proj directly: y = aT.T @ Wp.
Biases are zeros in setup_inputs(); bias matmuls are emitted only if nonzero.
"""

import sys

for p in ("/opt/trn_rl_repo",):
    if p not in sys.path:
        sys.path.insert(0, p)

import numpy as np
import ml_dtypes

import concourse.bass as bass
import concourse.tile as tile
from concourse import bacc, mybir
from concourse.bass_utils import run_bass_kernel_spmd

F32 = mybir.dt.float32
F32R = mybir.dt.float32r
BF16 = mybir.dt.bfloat16
EXPF = mybir.ActivationFunctionType.Exp
NP_BF16 = ml_dtypes.bfloat16

_CACHE = {}
BUILD_MARKS = []  # (label, n_instructions) snapshots for profiling tools


def ceil_div(a, b):
    return (a + b - 1) // b


def blob_layout(S, NX, with_attn_bias, with_proj_bias):
    """Offsets (in bf16 elements) of each logical tensor inside the single
    packed input blob. One input tensor instead of seven: each additional
    ExternalInput costs ~64us of per-execution launch overhead on this
    runtime (measured with chained executions of probe kernels)."""
    P = 128
    CW = min(512, S)
    segs = [
        ("x", S * NX),
        ("query", S * NX),
        ("c_attn_w", NX * 3 * NX),
        ("c_proj_w", NX * NX),
        ("ident", P * P),
        ("mask", P * P),
        ("ones", P * CW),
    ]
    if with_attn_bias:
        segs.append(("c_attn_b", 3 * NX))
    if with_proj_bias:
        segs.append(("c_proj_b", NX))
    offs, o = {}, 0
    for n, sz in segs:
        offs[n] = (o, sz)
        o += sz
    return offs, o


def build_module(S, NX, H, with_attn_bias, with_proj_bias, n_cores=8):
    """Build the per-core Bass module."""
    from contextlib import ExitStack

    HD = NX // H
    assert HD == 64, "kernel specialized for head_dim 64 (2 heads per 128-row chunk)"
    P = 128
    SB = S // P        # number of 128-row blocks of S
    KB = NX // P       # number of 128-deep contraction chunks over NX
    CW = min(512, S)   # column-tile width over S
    NQ = ceil_div(S, CW)
    NW = min(512, NX)  # column-tile width over NX
    NH = ceil_div(NX, NW)
    QW = min(256, NX)  # Wq streaming quarter width
    HC = HD + 1        # head stride in v_aug (v columns + ones column)
    scale = 1.0 / float(np.sqrt(HD))

    nc = bacc.Bacc("TRN2", target_bir_lowering=False, debug=False,
                   num_devices=n_cores)

    offs, total = blob_layout(S, NX, with_attn_bias, with_proj_bias)
    blob_d = nc.dram_tensor("blob", [total], BF16, kind="ExternalInput")

    def seg(name, *shape):
        o, sz = offs[name]
        assert int(np.prod(shape)) == sz, (name, shape, sz)
        if len(shape) == 2:
            return blob_d[o : o + sz].rearrange("(r c) -> r c", c=shape[1])
        return blob_d[o : o + sz].rearrange(
            "(k p c) -> p k c", p=shape[1], c=shape[2]
        )

    x_d = seg("x", S, NX)
    q_d = seg("query", S, NX)
    # [p, k, c] view of c_attn_w: row-block k, partition p, column c
    wa3_d = seg("c_attn_w", KB, P, 3 * NX)
    wp3_d = seg("c_proj_w", KB, P, NX)
    ident_d = seg("ident", P, P)
    mask_d = seg("mask", P, P)
    ones_d = seg("ones", P, CW)
    if with_attn_bias:
        ba_d = seg("c_attn_b", 1, 3 * NX)
    if with_proj_bias:
        bp_d = seg("c_proj_b", 1, NX)
    out_d = nc.dram_tensor("out", [S, NX], F32, kind="ExternalOutput")

    BUILD_MARKS.clear()

    def mark(label):
        BUILD_MARKS.append((label, len(nc.inst_map)))

    with tile.TileContext(nc) as tc, ExitStack() as top:
        consts = top.enter_context(tc.tile_pool(name="consts", bufs=1))
        ident = consts.tile([P, P], BF16, tag="ident")
        mask = consts.tile([P, P], BF16, tag="mask")
        ones = consts.tile([P, CW], BF16, tag="ones")
        nc.sync.dma_start(ident[:], ident_d[:])
        nc.sync.dma_start(mask[:], mask_d[:])
        nc.sync.dma_start(ones[:], ones_d[:])
        if with_attn_bias:
            ba = consts.tile([1, 3 * NX], BF16, tag="ba")
            nc.sync.dma_start(ba[:], ba_d[:])
        if with_proj_bias:
            bp = consts.tile([1, NX], BF16, tag="bp")
            nc.sync.dma_start(bp[:], bp_d[:])

        # kT and v_aug live from window 1 through window 2.
        qkv = top.enter_context(tc.tile_pool(name="qkv", bufs=1))
        kT = qkv.tile([P, KB * S], BF16, tag="kT")      # [NX, S]
        v_aug = qkv.tile([P, SB * H * HC], BF16, tag="v")
        qTin_pool = top.enter_context(tc.tile_pool(name="qTin", bufs=1))
        qTin = qTin_pool.tile([P, KB * S], BF16, tag="qTin")

        def transpose_into(src_d, r0, r1, dstT, nat_pool, tp_pool):
            # dstT[:, S*k + (sb-r0)*P ...] = src[P*sb : .., P*k : ..].T
            # 4 transposes share one PSUM tile -> one [128, 4*128] copy.
            dview = dstT.rearrange("p (k s) -> p k s", s=S)
            for sb in range(r0, r1):
                nat = nat_pool.tile([P, NX], BF16, tag="nat")
                nc.sync.dma_start(nat[:], src_d[P * sb : P * (sb + 1), :])
                for k0 in range(0, KB, 4):
                    ps = tp_pool.tile([P, 4 * P], BF16, tag="tp")
                    psv = ps.rearrange("p (k s) -> p k s", s=P)
                    for k in range(k0, k0 + 4):
                        nc.tensor.transpose(
                            psv[:, k - k0, :], nat[:, P * k : P * (k + 1)],
                            ident[:],
                        )
                    dst = dview[
                        :, k0 : k0 + 4, P * (sb - r0) : P * (sb - r0 + 1)
                    ]
                    if (sb + k0) % 8 < 4:
                        nc.vector.tensor_copy(dst, psv[:])
                    else:
                        nc.scalar.copy(dst, psv[:])

        # ---------------- Window 1: x side (kT, v) ----------------
        with ExitStack() as st_a:
            nat_pool = st_a.enter_context(tc.tile_pool(name="nat", bufs=3))
            tp_pool = st_a.enter_context(
                tc.tile_pool(name="tp", bufs=4, space="PSUM")
            )
            w_pool = st_a.enter_context(tc.tile_pool(name="w", bufs=2))
            pb_pool = st_a.enter_context(
                tc.tile_pool(name="pb", bufs=4, space="PSUM")
            )

            # ones columns of v_aug
            for sb in range(SB):
                va = v_aug[:, sb * H * HC : (sb + 1) * H * HC].rearrange(
                    "p (h c) -> p h c", c=HC
                )[:, :, HD : HD + 1]
                nc.vector.tensor_copy(
                    va, ones[:, :H].rearrange("p (h o) -> p h o", o=1)
                )
            mark("W1:setup")

            with tc.tile_pool(name="xT", bufs=1) as xT_pool:
                xT = xT_pool.tile([P, KB * S], BF16, tag="xT")
                transpose_into(x_d, 0, SB, xT, nat_pool, tp_pool)
                mark("W1:transposeX")

                # v in natural layout, scattered into v_aug
                for t in range(NH):
                    c0, c1 = NW * t, min(NW * (t + 1), NX)
                    w = w_pool.tile([P, KB * NW], BF16, tag="w")
                    for k in range(KB):
                        nc.sync.dma_start(
                            w[:, NW * k : NW * (k + 1)],
                            wa3_d[:, k, 2 * NX + c0 : 2 * NX + c1],
                        )
                    for sb in range(SB):
                        ps = pb_pool.tile([P, CW], F32, tag="pb")
                        for k in range(KB):
                            nc.tensor.matmul(
                                ps[:, : c1 - c0],
                                xT[:, S * k + P * sb : S * k + P * (sb + 1)],
                                w[:, NW * k : NW * k + (c1 - c0)],
                                start=(k == 0),
                                stop=(k == KB - 1) and not with_attn_bias,
                            )
                        if with_attn_bias:
                            nc.tensor.matmul(
                                ps[:, : c1 - c0],
                                ones[0:1, :P],
                                ba[:, 2 * NX + c0 : 2 * NX + c1],
                                start=False,
                                stop=True,
                            )
                        h0 = c0 // HD
                        nh = (c1 - c0) // HD
                        dst = v_aug[
                            :,
                            sb * H * HC + h0 * HC : sb * H * HC + (h0 + nh) * HC,
                        ].rearrange("p (h c) -> p h c", c=HC)[:, :, :HD]
                        srcv = ps[:, : c1 - c0].rearrange("p (h c) -> p h c", c=HD)
                        if (sb + t) % 2 == 0:
                            nc.vector.tensor_copy(dst, srcv)
                        else:
                            nc.scalar.copy(dst, srcv)
                mark("W1:projV")
                transpose_into(q_d, 0, SB, qTin, nat_pool, tp_pool)
                mark("W1:transposeQ")
                # kT = Wk.T @ xT, W streamed in column halves
                for half in range(NX // NW):
                    w = w_pool.tile([P, KB * NW], BF16, tag="w")
                    for k in range(KB):
                        nc.sync.dma_start(
                            w[:, NW * k : NW * (k + 1)],
                            wa3_d[:, k, NX + NW * half : NX + NW * (half + 1)],
                        )
                    for ml in range(NW // P):
                        m = half * (NW // P) + ml
                        for t in range(NQ):
                            c0, c1 = CW * t, min(CW * (t + 1), S)
                            ps = pb_pool.tile([P, CW], F32, tag="pb")
                            for k in range(KB):
                                nc.tensor.matmul(
                                    ps[:, : c1 - c0],
                                    w[:, NW * k + P * ml : NW * k + P * (ml + 1)],
                                    xT[:, S * k + c0 : S * k + c1],
                                    start=(k == 0),
                                    stop=(k == KB - 1) and not with_attn_bias,
                                )
                            if with_attn_bias:
                                nc.tensor.matmul(
                                    ps[:, : c1 - c0],
                                    ba[:, NX + P * m : NX + P * (m + 1)],
                                    ones[0:1, : c1 - c0],
                                    start=False,
                                    stop=True,
                                )
                            dst = kT[:, S * m + c0 : S * m + c1]
                            if (m + t) % 2 == 0:
                                nc.vector.tensor_copy(dst, ps[:, : c1 - c0])
                            else:
                                nc.scalar.copy(dst, ps[:, : c1 - c0])
                mark("W1:projK")

        # ------- Window 2: per sq half: projQ + attention -------
        aT_pool = top.enter_context(tc.tile_pool(name="aTp", bufs=1))
        aT = aT_pool.tile([P, KB * S], BF16, tag="aT")

        def attn_scores(h, qTmp, c0, c1, sc_pool, j, PT):
            ch = h // 2
            po = HD * (h % 2)
            lo = max(c0, P * j)
            ps = sc_pool.tile([P, CW], F32, tag="sc")
            nc.tensor.matmul(
                ps[:, lo - c0 : c1 - c0],
                kT[po : po + HD, S * ch + P * j : S * ch + P * (j + 1)],
                qTmp[po : po + HD, lo - c0 : c1 - c0],
                start=True,
                stop=True,
            )
            # exp over the exact causal region
            nc.scalar.activation(
                PT[:, CW * j + (lo - c0) : CW * j + (c1 - c0)],
                ps[:, lo - c0 : c1 - c0],
                EXPF,
                scale=scale,
            )
            if lo == P * j:
                # diagonal block: zero strictly-lower (sk>sq)
                d0 = CW * j + (lo - c0)
                nc.vector.tensor_mul(
                    PT[:, d0 : d0 + P], PT[:, d0 : d0 + P], mask[:]
                )

        def attn_tail(h, c0, c1, PT, at_pool, nrm_pool):
            ch = h // 2
            po = HD * (h % 2)
            W = c1 - c0
            jmax = min(SB, ceil_div(c1, P))
            # attnT_aug = [v_h | 1].T @ P : rows 0..HD attn, row HD = l
            psA = at_pool.tile([P, CW], F32, tag="at")
            for j in range(jmax):
                lo = max(c0, P * j)
                nc.tensor.matmul(
                    psA[:HC, lo - c0 : W],
                    v_aug[:, (j * H + h) * HC : (j * H + h + 1) * HC],
                    PT[:, CW * j + (lo - c0) : CW * j + (c1 - c0)],
                    start=(j == 0),
                    stop=(j == jmax - 1),
                )
            rec = nrm_pool.tile([P, CW], F32, tag="rec")
            nc.vector.reciprocal(rec[HD : HD + 1, :W], psA[HD : HD + 1, :W])
            # hop 1/l to partition 0 (SBUF-to-SBUF DMA crosses partitions;
            # gpsimd broadcast only reads partition 0)
            nc.sync.dma_start(rec[0:1, :W], rec[HD : HD + 1, :W])
            bcs = nrm_pool.tile([P, CW], F32, tag="bcs")
            nc.gpsimd.partition_broadcast(bcs[:HD, :W], rec[0:1, :W])
            if po == 0:
                nc.vector.tensor_mul(
                    aT[:HD, S * ch + c0 : S * ch + c1],
                    psA[:HD, :W],
                    bcs[:HD, :W],
                )
            else:
                ash = nrm_pool.tile([P, CW], BF16, tag="ash")
                nc.vector.tensor_mul(ash[:HD, :W], psA[:HD, :W], bcs[:HD, :W])
                # partition shift 0->64 via SBUF-to-SBUF DMA
                nc.sync.dma_start(
                    aT[po : po + HD, S * ch + c0 : S * ch + c1], ash[:HD, :W]
                )

        def attn_pair(m, qTmp, c0, c1, pt_pool, sc_pool, at_pool, nrm_pool):
            # Interleave the two heads' K=64 score matmuls so they sit in
            # adjacent PE queue slots with disjoint row groups (partitions
            # 0-63 vs 64-127) -> the systolic array runs them concurrently.
            jmax = min(SB, ceil_div(c1, P))
            PTa = pt_pool.tile([P, SB * CW], BF16, tag="pt")
            PTb = pt_pool.tile([P, SB * CW], BF16, tag="pt")
            for j in range(jmax):
                attn_scores(2 * m, qTmp, c0, c1, sc_pool, j, PTa)
                attn_scores(2 * m + 1, qTmp, c0, c1, sc_pool, j, PTb)
            attn_tail(2 * m, c0, c1, PTa, at_pool, nrm_pool)
            attn_tail(2 * m + 1, c0, c1, PTb, at_pool, nrm_pool)

        with tc.tile_pool(name="wq", bufs=1) as wq_pool, tc.tile_pool(
            name="qTmp", bufs=3
        ) as qTmp_pool, tc.tile_pool(
            name="pbq", bufs=2, space="PSUM"
        ) as pbq_pool, tc.tile_pool(name="pt", bufs=3) as pt_pool, tc.tile_pool(
            name="sc", bufs=4, space="PSUM"
        ) as sc_pool, tc.tile_pool(
            name="at", bufs=2, space="PSUM"
        ) as at_pool, tc.tile_pool(name="nrm", bufs=2) as nrm_pool:
            for qtr in range(NX // QW):
                wq = wq_pool.tile([P, KB * QW], BF16, tag="wq")
                for k in range(KB):
                    nc.sync.dma_start(
                        wq[:, QW * k : QW * (k + 1)],
                        wa3_d[:, k, QW * qtr : QW * (qtr + 1)],
                    )
                for ml in range(QW // P):
                    m = qtr * (QW // P) + ml
                    # t-inner: interleave the light (chain-bound) low-sq
                    # units with the heavy high-sq units for better packing,
                    # and load each Wq quarter once instead of once per half.
                    for t in range(NQ):
                        c0, c1 = CW * t, min(CW * (t + 1), S)
                        Wt = c1 - c0
                        ps = pbq_pool.tile([P, CW], F32, tag="pbq")
                        for k in range(KB):
                            nc.tensor.matmul(
                                ps[:, :Wt],
                                wq[:, QW * k + P * ml : QW * k + P * (ml + 1)],
                                qTin[:, S * k + c0 : S * k + c1],
                                start=(k == 0),
                                stop=(k == KB - 1) and not with_attn_bias,
                            )
                        if with_attn_bias:
                            nc.tensor.matmul(
                                ps[:, :Wt],
                                ba[:, P * m : P * (m + 1)],
                                ones[0:1, :Wt],
                                start=False,
                                stop=True,
                            )
                        qTmp = qTmp_pool.tile([P, CW], BF16, tag="qTmp")
                        if (m + t) % 2 == 0:
                            nc.vector.tensor_copy(qTmp[:, :Wt], ps[:, :Wt])
                        else:
                            nc.scalar.copy(qTmp[:, :Wt], ps[:, :Wt])
                        attn_pair(
                            m, qTmp, c0, c1,
                            pt_pool, sc_pool, at_pool, nrm_pool,
                        )
                mark(f"W2:qtr{qtr}")

        # ---------------- Phase D: output projection ----------------
        with tc.tile_pool(name="wp", bufs=1) as wp_pool, tc.tile_pool(
            name="pd", bufs=4, space="PSUM"
        ) as pd_pool, tc.tile_pool(name="yo", bufs=3) as yo_pool:
            wp = wp_pool.tile([P, KB * NX], BF16, tag="wp")
            for k in range(KB):
                nc.sync.dma_start(
                    wp[:, NX * k : NX * (k + 1)], wp3_d[:, k, :]
                )
            for sb in range(SB):
                for t in range(NH):
                    c0, c1 = NW * t, min(NW * (t + 1), NX)
                    ps = pd_pool.tile([P, NW], F32, tag="pd")
                    for k in range(KB):
                        nc.tensor.matmul(
                            ps[:, : c1 - c0],
                            aT[:, S * k + P * sb : S * k + P * (sb + 1)],
                            wp[:, NX * k + c0 : NX * k + c1],
                            start=(k == 0),
                            stop=(k == KB - 1) and not with_proj_bias,
                        )
                    if with_proj_bias:
                        nc.tensor.matmul(
                            ps[:, : c1 - c0],
                            ones[0:1, :P],
                            bp[:, c0:c1],
                            start=False,
                            stop=True,
                        )
                    yo = yo_pool.tile([P, NW], F32, tag="yo")
                    if (sb + t) % 2 == 0:
                        nc.vector.tensor_copy(yo[:, : c1 - c0], ps[:, : c1 - c0])
                    else:
                        nc.scalar.copy(yo[:, : c1 - c0], ps[:, : c1 - c0])
                    nc.sync.dma_start(
                        out_d[P * sb : P * (sb + 1), c0:c1], yo[:, : c1 - c0]
                    )
        mark("D:proj")

    nc.compile()
    return nc


def get_module(S, NX, H, with_attn_bias, with_proj_bias, n_cores=8):
    key = (S, NX, H, with_attn_bias, with_proj_bias, n_cores)
    if key not in _CACHE:
        _CACHE[key] = build_module(
            S, NX, H, with_attn_bias, with_proj_bias, n_cores
        )
    return _CACHE[key]


def make_const_inputs(S, NX):
    P = 128
    CW = min(512, S)
    return {
        "ident": np.eye(P, dtype=NP_BF16),
        # mask[sk, sq] = 1 where sk <= sq (upper triangular incl diagonal)
        "mask": np.triu(np.ones((P, P), dtype=NP_BF16)),
        "ones": np.ones((P, CW), dtype=NP_BF16),
    }


def make_in_maps(x, query, c_attn_w, c_proj_w, c_attn_b=None, c_proj_b=None,
                 n_cores=8):
    """Pack the per-core single-blob inputs. All array arguments must already
    be bf16 (NP_BF16); x/query are [B, S, NX], weights 2D."""
    B, S, NX = x.shape
    consts = make_const_inputs(S, NX)
    tail = [
        np.ascontiguousarray(c_attn_w).ravel(),
        np.ascontiguousarray(c_proj_w).ravel(),
        consts["ident"].ravel(),
        consts["mask"].ravel(),
        consts["ones"].ravel(),
    ]
    if c_attn_b is not None:
        tail.append(np.ascontiguousarray(c_attn_b).ravel())
    if c_proj_b is not None:
        tail.append(np.ascontiguousarray(c_proj_b).ravel())
    tail = np.concatenate(tail)
    offs, total = blob_layout(
        S, NX, c_attn_b is not None, c_proj_b is not None
    )
    in_maps = []
    for c in range(n_cores):
        blob = np.concatenate(
            [x[c % B].ravel(), query[c % B].ravel(), tail]
        )
        assert blob.size == total
        in_maps.append({"blob": blob})
    return in_maps


def kernel(x, query, c_attn_w, c_attn_b, c_proj_w, c_proj_b, _trace=False):
    x = np.asarray(x, dtype=np.float32).astype(NP_BF16)
    query = np.asarray(query, dtype=np.float32).astype(NP_BF16)
    c_attn_w = np.asarray(c_attn_w, dtype=np.float32).astype(NP_BF16)
    c_attn_b = np.asarray(c_attn_b, dtype=np.float32)
    c_proj_w = np.asarray(c_proj_w, dtype=np.float32).astype(NP_BF16)
    c_proj_b = np.asarray(c_proj_b, dtype=np.float32)

    B, S, NX = x.shape
    H = 16
    wab = bool(np.any(c_attn_b != 0))
    wpb = bool(np.any(c_proj_b != 0))
    n_cores = 8
    nc = get_module(S, NX, H, wab, wpb, n_cores)

    in_maps = make_in_maps(
        x, query, c_attn_w, c_proj_w,
        c_attn_b.reshape(1, -1).astype(NP_BF16) if wab else None,
        c_proj_b.reshape(1, -1).astype(NP_BF16) if wpb else None,
        n_cores=n_cores,
    )

    res = run_bass_kernel_spmd(
        nc, in_maps, core_ids=list(range(n_cores)), trace=_trace
    )
    out = np.stack([res.results[c]["out"] for c in range(B)], axis=0)
    if _trace:
        kernel._last_results = res
    return out
